# revision 59
# baseline (speedup 1.0000x reference)
"""CSSR classifier kernel for 8 Trainium2 NeuronCores.

Math (per class k):
    h1 = tanh(W1[k] @ xf)          xf: [C=512, B*P=4096]
    h2 = tanh(W2[k] @ h1)
    lt = tanh(W3[k] @ h2)          [L=32, B*P]
    er_raw  = sum_l (lt - proto )^2     -> [B*P]
    er_raw1 = sum_l (lt - proto1)^2
Device returns er_raw per class; host applies  er = max(-0.1*er_raw, -100),
assembles logits and computes the scalar pull/push losses (O(K*B) work).

Sharding: class dim K=100 -> 8 cores x 13 class slots (104, last 4 dummy).
x is replicated; each core holds only its slice of W1/W2/W3/prototypes.
"""

import contextlib
import os
import sys

if "/opt/trn_rl_repo" not in sys.path:
    sys.path.insert(0, "/opt/trn_rl_repo")

import numpy as np
import ml_dtypes

import concourse.bacc as bacc
import concourse.mybir as mybir
import concourse.tile as tile
from concourse import bass_utils

BF16 = mybir.dt.bfloat16
F32 = mybir.dt.float32
Tanh = mybir.ActivationFunctionType.Tanh

# problem dims (hardcoded per contract)
B, C, H, W = 64, 512, 8, 8
P = H * W                  # 64 spatial positions
K = 100                    # classes
H1, HID, L = 64, 128, 32
CLIP = 100.0
RED = -0.1
PUSH_THRESH = 10000.0

NCORES = 8
KC = 13                    # class slots per core (8*13 = 104 >= 100)
NPAIR = 7                  # stage-1 class pairs per core (14 slots, #13 dummy)
NB = B * P                 # 4096 free columns
CHUNK = 1024
NCHUNK = NB // CHUNK       # 4

# groups of <=4 classes that share one stage-3/er pack
GROUPS = [
    ([0, 1], [0, 1, 2, 3]),
    ([2, 3], [4, 5, 6, 7]),
    ([4, 5], [8, 9, 10, 11]),
    ([6], [12]),
]

_CACHE = {}
PIPELINE = False


def _build_program(reps=1):
    """reps>1 wraps the compute in a hardware loop (timing builds only)."""
    nc = bacc.Bacc("TRN2", target_bir_lowering=False, debug=False)

    xf_d = nc.dram_tensor("xf", [C, NB], BF16, kind="ExternalInput").ap()
    w1_d = nc.dram_tensor("w1p", [NPAIR, C, 2 * H1], BF16, kind="ExternalInput").ap()
    # W2[k].T duplicated vertically so lhsT can be based at partition 0 or 64
    # (matmul requires lhsT and rhs to share a base partition)
    w2_d = nc.dram_tensor("w2t", [KC, 2 * H1, HID], BF16, kind="ExternalInput").ap()
    w3_d = nc.dram_tensor("w3t", [KC, HID, L], BF16, kind="ExternalInput").ap()
    # prototypes pre-broadcast over the 16 b's of a chunk: [2, group, 128, CHUNK]
    pr_d = nc.dram_tensor("prb", [2, 4, 128, CHUNK], BF16, kind="ExternalInput").ap()
    # block "ones" with 16x output replication: ones[l, m] = (l//32 == m//16).
    # The er-reduce matmul then fills all 64 output partitions, so the
    # scale+clip PSUM->SBUF copy is one full-width DVE op per chunk.
    ones_d = nc.dram_tensor("onesb", [128, 64], BF16, kind="ExternalInput").ap()
    er0_d = nc.dram_tensor("er0", [KC, NB], F32, kind="ExternalOutput").ap()
    er1_d = nc.dram_tensor("er1", [KC, NB], F32, kind="ExternalOutput").ap()

    with tile.TileContext(nc) as tc:
        with (
            tc.tile_pool(name="weights", bufs=1) as wpool,
            tc.tile_pool(name="xfp", bufs=1) as xfp,
            tc.tile_pool(name="h1p", bufs=6) as h1pool,
            tc.tile_pool(name="h2p", bufs=8) as h2pool,
            tc.tile_pool(name="ltp", bufs=4) as ltpool,
            tc.tile_pool(name="sqp", bufs=6) as sqpool,
            tc.tile_pool(name="ersp", bufs=3) as erspool,
            tc.tile_pool(name="mmp", bufs=2, space="PSUM") as mmpool,
            tc.tile_pool(name="s3p", bufs=1, space="PSUM") as s3pool,
            tc.tile_pool(name="erp", bufs=1, space="PSUM") as erpool,
        ):
            # ---- DMA issue order matters: the first matmul needs w1[pair0]
            # and the n=0 xf chunk, so those go first; the rest of the
            # weights trail behind, interleaved group-by-group ----
            def load_w1_pair(p):
                tiles = []
                for kk in range(4):
                    t = wpool.tile([128, 2 * H1], BF16, tag=f"w1_{p}_{kk}",
                                   name=f"w1_{p}_{kk}")
                    nc.sync.dma_start(
                        out=t, in_=w1_d[p, 128 * kk:128 * (kk + 1), :])
                    tiles.append(t)
                return tiles

            def load_xf_chunk(n):
                tiles = []
                for kk in range(4):
                    t = xfp.tile([128, CHUNK], BF16, tag=f"xf_{kk}_{n}",
                                 name=f"xf_{kk}_{n}")
                    nc.sync.dma_start(
                        out=t,
                        in_=xf_d[128 * kk:128 * (kk + 1),
                                 CHUNK * n:CHUNK * (n + 1)])
                    tiles.append(t)
                return tiles

            w1t = [None] * NPAIR
            w2t = [None] * KC
            w3t = [None] * KC
            prt = [[None] * 4 for _ in range(2)]
            xft_n = [None] * NCHUNK  # xft_n[n][kk]

            def load_w23(classes):
                for k in classes:
                    t = wpool.tile([2 * H1, HID], BF16, tag=f"w2_{k}",
                                   name=f"w2_{k}")
                    nc.sync.dma_start(out=t, in_=w2_d[k])
                    w2t[k] = t
                    t = wpool.tile([HID, L], BF16, tag=f"w3_{k}", name=f"w3_{k}")
                    nc.sync.dma_start(out=t, in_=w3_d[k])
                    w3t[k] = t

            def load_pr(g):
                for j in range(2):
                    t = wpool.tile([128, CHUNK], BF16, tag=f"pr_{j}_{g}",
                                   name=f"pr_{j}_{g}")
                    nc.sync.dma_start(out=t, in_=pr_d[j, g])
                    prt[j][g] = t

            def load_group_weights(g, pairs, classes):
                for p in pairs:
                    if w1t[p] is None:
                        w1t[p] = load_w1_pair(p)
                load_w23(classes)
                load_pr(g)

            ones_t = wpool.tile([128, 64], BF16, tag="ones", name="ones_t")
            w1t[0] = load_w1_pair(0)
            xft_n[0] = load_xf_chunk(0)
            w1t[1] = load_w1_pair(1)
            load_w23(GROUPS[0][1])
            nc.sync.dma_start(out=ones_t, in_=ones_d)
            xft_n[1] = load_xf_chunk(1)
            load_pr(0)
            load_group_weights(1, GROUPS[1][0], GROUPS[1][1])
            xft_n[2] = load_xf_chunk(2)
            xft_n[3] = load_xf_chunk(3)
            load_group_weights(2, GROUPS[2][0], GROUPS[2][1])
            load_group_weights(3, GROUPS[3][0], GROUPS[3][1])
            xft = [[xft_n[n][kk] for n in range(NCHUNK)] for kk in range(4)]

            # ---- main loop ----
            # Software pipeline: chunk n emits its stage-1/2 work with chunk
            # n-1's stage-3/er work interleaved between the stage-2 matmuls,
            # so PE has independent filler during the ACT-gated PSUM waits.
            def emit_back(g, np_, h2s_prev, er_s):
                """stage-3 + er path for chunk np_ using h2s_prev (list of
                per-class h2 tiles). Returns a generator-like list of
                closures so the caller can interleave them."""
                pairs, classes = GROUPS[g]
                nr = 32 * len(classes)
                mr = 16 * len(classes)
                ps3 = s3pool.tile([128, CHUNK], F32, tag="s3p",
                                  name=f"ps3_{g}_{np_}")

                def s3_for(ci):
                    cls = classes[ci]
                    for h in range(2):
                        nc.tensor.matmul(
                            ps3[32 * ci:32 * (ci + 1), 512 * h:512 * (h + 1)],
                            w3t[cls],
                            h2s_prev[ci][:, 512 * h:512 * (h + 1)],
                            start=True, stop=True,
                            tile_position=(0, 32 * ci))

                def finish():
                    lt = ltpool.tile([128, CHUNK], BF16, tag="lt",
                                     name=f"lt_{g}_{np_}")
                    nc.scalar.activation(lt[0:nr, :], ps3[0:nr, :], Tanh)
                    er_ps = erpool.tile([128, CHUNK], F32, tag="erp",
                                        name=f"erps_{g}_{np_}")
                    for j in range(2):
                        d = sqpool.tile([128, CHUNK], BF16, tag="sq",
                                        name=f"d_{g}_{np_}_{j}")
                        nc.vector.tensor_sub(d[0:nr, :], lt[0:nr, :],
                                             prt[j][g][0:nr, :])
                        sq = sqpool.tile([128, CHUNK], BF16, tag="sq",
                                         name=f"sq_{g}_{np_}_{j}")
                        nc.vector.tensor_mul(sq[0:nr, :], d[0:nr, :],
                                             d[0:nr, :])
                        for h in range(2):
                            nc.tensor.matmul(
                                er_ps[64 * j:64 * j + mr,
                                      512 * h:512 * (h + 1)],
                                ones_t[0:nr, 0:mr],
                                sq[0:nr, 512 * h:512 * (h + 1)],
                                start=True, stop=True,
                                tile_position=(0, 64 * j))
                    # er = max(raw * RED, -CLIP) fused into the PSUM->SBUF copy
                    if nr == 128:
                        nc.vector.tensor_scalar(
                            er_s[:, CHUNK * np_:CHUNK * (np_ + 1)],
                            er_ps[:, :],
                            RED, -CLIP,
                            op0=mybir.AluOpType.mult, op1=mybir.AluOpType.max)
                    else:
                        for j in range(2):
                            nc.vector.tensor_scalar(
                                er_s[64 * j:64 * j + mr,
                                     CHUNK * np_:CHUNK * (np_ + 1)],
                                er_ps[64 * j:64 * j + mr, :],
                                RED, -CLIP,
                                op0=mybir.AluOpType.mult, op1=mybir.AluOpType.max)
                    # per-chunk output DMA keeps the tail short
                    cs = slice(CHUNK * np_, CHUNK * (np_ + 1))
                    nc.sync.dma_start(
                        out=er0_d[4 * g:4 * g + len(classes), cs],
                        in_=er_s[0:16 * len(classes):16, cs])
                    nc.sync.dma_start(
                        out=er1_d[4 * g:4 * g + len(classes), cs],
                        in_=er_s[64:64 + 16 * len(classes):16, cs])

                return s3_for, finish

            def chunk_front(g, n):
                """stage-1 + stage-2 for chunk n; returns h2 tiles."""
                pairs, classes = GROUPS[g]
                h1s = []
                for pair in pairs:
                    ps1 = mmpool.tile([128, CHUNK], F32, tag="mm",
                                      name=f"ps1_{g}_{n}_{pair}")
                    for kk in range(4):
                        for h in range(2):
                            nc.tensor.matmul(
                                ps1[:, 512 * h:512 * (h + 1)],
                                w1t[pair][kk],
                                xft[kk][n][:, 512 * h:512 * (h + 1)],
                                start=(kk == 0), stop=(kk == 3))
                    h1 = h1pool.tile([128, CHUNK], BF16, tag="h1",
                                     name=f"h1_{g}_{n}_{pair}")
                    nc.scalar.activation(h1, ps1, Tanh)
                    h1s.append(h1)
                return h1s

            def s2_class(g, n, ci, h1s):
                pairs, classes = GROUPS[g]
                cls = classes[ci]
                ps2 = mmpool.tile([128, CHUNK], F32, tag="mm",
                                  name=f"ps2_{g}_{n}_{ci}")
                hsrc = h1s[ci // 2]
                off = H1 * (ci % 2)
                for h in range(2):
                    nc.tensor.matmul(
                        ps2[:, 512 * h:512 * (h + 1)],
                        w2t[cls][off:off + H1, :],
                        hsrc[off:off + H1, 512 * h:512 * (h + 1)],
                        start=True, stop=True)
                h2 = h2pool.tile([128, CHUNK], BF16, tag="h2",
                                 name=f"h2_{g}_{n}_{ci}")
                nc.scalar.activation(h2, ps2, Tanh)
                return h2

            # rows 0:64 of er_s = er (proto0, class ci at row 16*ci), rows
            # 64:128 = er1 (proto1); 16x row replication from the
            # ones-matmul, only every 16th row is DMA'd out
            loop_cm = (tc.For_i(0, reps, 1,
                                hint_engines=(mybir.EngineType.PE,
                                              mybir.EngineType.Activation,
                                              mybir.EngineType.DVE,
                                              mybir.EngineType.SP))
                       if reps > 1 else contextlib.nullcontext())
            with loop_cm:
              for phase in ([0, 1], [2, 3]):
                ers = {g: erspool.tile([128, NB], F32, tag="ers",
                                       name=f"ers_{g}")
                       for g in phase}
                if PIPELINE:
                    pend = {g: None for g in phase}  # (h2s, n) awaiting back
                    for n in range(NCHUNK + 1):
                        for g in phase:
                            classes = GROUPS[g][1]
                            back = None
                            if pend[g] is not None:
                                h2s_prev, np_ = pend[g]
                                back = emit_back(g, np_, h2s_prev, ers[g])
                            if n < NCHUNK:
                                h1s = chunk_front(g, n)
                                h2s = []
                                for ci in range(len(classes)):
                                    h2s.append(s2_class(g, n, ci, h1s))
                                    if back is not None:
                                        back[0](ci)  # interleave prev s3
                                if back is not None:
                                    back[1]()
                                pend[g] = (h2s, n)
                            else:
                                if back is not None:
                                    for ci in range(len(classes)):
                                        back[0](ci)
                                    back[1]()
                                pend[g] = None
                else:
                    for n in range(NCHUNK):
                        for g in phase:
                            classes = GROUPS[g][1]
                            h1s = chunk_front(g, n)
                            h2s = [s2_class(g, n, ci, h1s)
                                   for ci in range(len(classes))]
                            back = emit_back(g, n, h2s, ers[g])
                            for ci in range(len(classes)):
                                back[0](ci)
                            back[1]()

    nc.compile()
    return nc


def _prep_in_maps(x, W1, W2, W3, prototypes, prototypes1):
    bf16 = ml_dtypes.bfloat16
    KPAD = NCORES * KC

    x = np.asarray(x, np.float32)
    xf = np.ascontiguousarray(
        x.reshape(B, C, P).transpose(1, 0, 2).reshape(C, NB)).astype(bf16)

    def pad_k(a):
        out = np.zeros((KPAD,) + a.shape[1:], np.float32)
        out[:K] = np.asarray(a, np.float32)
        return out

    W1p = pad_k(W1)                       # [104, H1, C]
    W2p = pad_k(W2)                       # [104, HID, H1]
    W3p = pad_k(W3)                       # [104, L, HID]
    Pr0 = pad_k(np.asarray(prototypes, np.float32).reshape(K, L, P))
    Pr1 = pad_k(np.asarray(prototypes1, np.float32).reshape(K, L, P))

    ones_blk = np.zeros((128, 64), bf16)
    for m in range(64):
        ones_blk[32 * (m // 16):32 * (m // 16) + 32, m] = 1.0

    in_maps = []
    for c in range(NCORES):
        s = slice(c * KC, (c + 1) * KC)
        w1c = W1p[s].transpose(0, 2, 1)   # [13, C, H1]
        w1c = np.concatenate([w1c, np.zeros((1, C, H1), np.float32)], 0)
        w1pair = np.ascontiguousarray(
            w1c.reshape(NPAIR, 2, C, H1).transpose(0, 2, 1, 3)
            .reshape(NPAIR, C, 2 * H1)).astype(bf16)
        w2c = np.ascontiguousarray(W2p[s].transpose(0, 2, 1)).astype(bf16)
        w2c = np.concatenate([w2c, w2c], axis=1)          # [13, 128, HID]
        w3c = np.ascontiguousarray(W3p[s].transpose(0, 2, 1)).astype(bf16)
        prc = np.zeros((2, 4, 128, CHUNK), np.float32)
        for src, j in ((Pr0[s], 0), (Pr1[s], 1)):
            for g in range(4):
                for jj in range(4):
                    ks = 4 * g + jj
                    if ks < KC:
                        # tile the [L, P] pattern across the 16 b's of a chunk
                        prc[j, g, 32 * jj:32 * (jj + 1)] = np.tile(
                            src[ks], (1, CHUNK // P))
        in_maps.append({
            "xf": xf,
            "w1p": w1pair,
            "w2t": w2c,
            "w3t": w3c,
            "prb": prc.astype(bf16),
            "onesb": ones_blk,
        })
    return in_maps


def _assemble(results, ycls):
    # device already applied  er = max(raw * RED, -CLIP)
    er0 = np.concatenate([r["er0"] for r in results], 0)[:K]   # [100, 4096]
    er1 = np.concatenate([r["er1"] for r in results], 0)[:K]

    logits = np.ascontiguousarray(
        er0.reshape(K, B, P).transpose(1, 0, 2).reshape(B, K, H, W))
    logits1 = np.ascontiguousarray(
        er1.reshape(K, B, P).transpose(1, 0, 2).reshape(B, K, H, W))

    f = er0.reshape(K, B, P).sum(axis=2, dtype=np.float32)     # [K, B]
    f1 = er1.reshape(K, B, P).sum(axis=2, dtype=np.float32)
    ycls = np.asarray(ycls)
    mask_eq = (ycls[None, :] == np.arange(K)[:, None]).astype(np.float32)
    n_eq = mask_eq.sum(axis=1)
    pull = np.where(n_eq > 0, (f1 * mask_eq).sum(axis=1) / np.maximum(n_eq, 1.0),
                    0.0).sum(dtype=np.float32)
    comb = (1.0 - mask_eq) * (f < PUSH_THRESH)
    n_comb = comb.sum(axis=1)
    push = np.where(n_comb > 0, (f * comb).sum(axis=1) / np.maximum(n_comb, 1.0),
                    0.0).sum(dtype=np.float32)
    return logits, logits1, np.float32(pull), np.float32(push)


def kernel_ex(inputs, trace=False):
    """Run the bass kernel; returns ((logits, logits1, pull, push), exec_time_ns)."""
    nc = _CACHE.get("nc")
    if nc is None:
        nc = _build_program()
        _CACHE["nc"] = nc
    in_maps = _prep_in_maps(inputs["x"], inputs["W1"], inputs["W2"],
                            inputs["W3"], inputs["prototypes"],
                            inputs["prototypes1"])
    try:
        res = bass_utils.run_bass_kernel_spmd(
            nc, in_maps, core_ids=list(range(NCORES)), trace=trace)
    except ModuleNotFoundError:
        # BASS_TRACE in the env but no axon NTFF hook module available here
        os.environ["BASS_NEVER_TRACE"] = "1"
        res = bass_utils.run_bass_kernel_spmd(
            nc, in_maps, core_ids=list(range(NCORES)), trace=False)
    outs = _assemble(res.results, inputs["ycls"])
    return outs, res.exec_time_ns


def kernel(**inputs):
    outs, _ = kernel_ex(inputs, trace=False)
    return outs


# revision 68
# speedup vs baseline: 1.1563x; 1.1563x over previous
"""CSSR classifier kernel for 8 Trainium2 NeuronCores.

Math (per class k):
    h1 = tanh(W1[k] @ xf)          xf: [C=512, B*P=4096]
    h2 = tanh(W2[k] @ h1)
    lt = tanh(W3[k] @ h2)          [L=32, B*P]
    er_raw  = sum_l (lt - proto )^2     -> [B*P]
    er_raw1 = sum_l (lt - proto1)^2
Device returns er_raw per class; host applies  er = max(-0.1*er_raw, -100),
assembles logits and computes the scalar pull/push losses (O(K*B) work).

Sharding: class dim K=100 -> 8 cores x 13 class slots (104, last 4 dummy).
x is replicated; each core holds only its slice of W1/W2/W3/prototypes.
"""

import contextlib
import os
import sys

if "/opt/trn_rl_repo" not in sys.path:
    sys.path.insert(0, "/opt/trn_rl_repo")

import numpy as np
import ml_dtypes

import concourse.bacc as bacc
import concourse.mybir as mybir
import concourse.tile as tile
from concourse import bass_utils

BF16 = mybir.dt.bfloat16
F32 = mybir.dt.float32
Tanh = mybir.ActivationFunctionType.Tanh

# problem dims (hardcoded per contract)
B, C, H, W = 64, 512, 8, 8
P = H * W                  # 64 spatial positions
K = 100                    # classes
H1, HID, L = 64, 128, 32
CLIP = 100.0
RED = -0.1
PUSH_THRESH = 10000.0

NCORES = 8
# Load balance: 100 = 8*12.5. Each core gets 12 full classes; the 4 leftover
# classes (96..99) are batch-split, half (B=32 -> 2048 columns) on each of
# two cores. Slot 12 of every core is its half-width leftover class.
KF = 12                    # full classes per core
KC = 13                    # weight slots per core (12 full + 1 half)
NPAIR = 7                  # stage-1 pairs (pairs 0-5 full, pair 6 = leftover+dummy)
NB = B * P                 # 4096 free columns
CHUNK = 1024
NCHUNK = NB // CHUNK       # 4
NH = NB // 2               # 2048 columns for the half class
NCHUNK_H = NH // CHUNK     # 2

# groups of <=4 classes that share one stage-3/er pack; group 3 is the
# half-width leftover class
GROUPS = [
    ([0, 1], [0, 1, 2, 3]),
    ([2, 3], [4, 5, 6, 7]),
    ([4, 5], [8, 9, 10, 11]),
    ([6], [12]),
]
G_NCH = [NCHUNK, NCHUNK, NCHUNK, NCHUNK_H]  # chunks per group

_CACHE = {}
PIPELINE = False


def _build_program(reps=1):
    """reps>1 wraps the compute in a hardware loop (timing builds only)."""
    nc = bacc.Bacc("TRN2", target_bir_lowering=False, debug=False)

    xf_d = nc.dram_tensor("xf", [C, NB], BF16, kind="ExternalInput").ap()
    # per-core half of xf for this core's leftover class (its batch half)
    xfh_d = nc.dram_tensor("xfh", [C, NH], BF16, kind="ExternalInput").ap()
    w1_d = nc.dram_tensor("w1p", [NPAIR, C, 2 * H1], BF16, kind="ExternalInput").ap()
    # W2[k].T duplicated vertically so lhsT can be based at partition 0 or 64
    # (matmul requires lhsT and rhs to share a base partition)
    w2_d = nc.dram_tensor("w2t", [KC, 2 * H1, HID], BF16, kind="ExternalInput").ap()
    w3_d = nc.dram_tensor("w3t", [KC, HID, L], BF16, kind="ExternalInput").ap()
    # prototypes pre-broadcast over the 16 b's of a chunk: [2, group, 128, CHUNK]
    pr_d = nc.dram_tensor("prb", [2, 4, 128, CHUNK], BF16, kind="ExternalInput").ap()
    # block "ones" with 16x output replication: ones[l, m] = (l//32 == m//16).
    # The er-reduce matmul then fills all 64 output partitions, so the
    # scale+clip PSUM->SBUF copy is one full-width DVE op per chunk.
    ones_d = nc.dram_tensor("onesb", [128, 64], BF16, kind="ExternalInput").ap()
    er0_d = nc.dram_tensor("er0", [KF, NB], F32, kind="ExternalOutput").ap()
    er1_d = nc.dram_tensor("er1", [KF, NB], F32, kind="ExternalOutput").ap()
    erh0_d = nc.dram_tensor("erh0", [1, NH], F32, kind="ExternalOutput").ap()
    erh1_d = nc.dram_tensor("erh1", [1, NH], F32, kind="ExternalOutput").ap()

    with tile.TileContext(nc) as tc:
        with (
            tc.tile_pool(name="weights", bufs=1) as wpool,
            tc.tile_pool(name="xfp", bufs=1) as xfp,
            tc.tile_pool(name="h1p", bufs=6) as h1pool,
            tc.tile_pool(name="h2p", bufs=8) as h2pool,
            tc.tile_pool(name="ltp", bufs=4) as ltpool,
            tc.tile_pool(name="sqp", bufs=6) as sqpool,
            tc.tile_pool(name="ersp", bufs=3) as erspool,
            tc.tile_pool(name="mmp", bufs=2, space="PSUM") as mmpool,
            tc.tile_pool(name="s3p", bufs=1, space="PSUM") as s3pool,
            tc.tile_pool(name="erp", bufs=1, space="PSUM") as erpool,
        ):
            # ---- DMA issue order matters: the first matmul needs w1[pair0]
            # and the n=0 xf chunk, so those go first; the rest of the
            # weights trail behind, interleaved group-by-group ----
            def load_w1_pair(p):
                tiles = []
                for kk in range(4):
                    t = wpool.tile([128, 2 * H1], BF16, tag=f"w1_{p}_{kk}",
                                   name=f"w1_{p}_{kk}")
                    nc.sync.dma_start(
                        out=t, in_=w1_d[p, 128 * kk:128 * (kk + 1), :])
                    tiles.append(t)
                return tiles

            def load_xf_chunk(n):
                tiles = []
                for kk in range(4):
                    t = xfp.tile([128, CHUNK], BF16, tag=f"xf_{kk}_{n}",
                                 name=f"xf_{kk}_{n}")
                    nc.sync.dma_start(
                        out=t,
                        in_=xf_d[128 * kk:128 * (kk + 1),
                                 CHUNK * n:CHUNK * (n + 1)])
                    tiles.append(t)
                return tiles

            w1t = [None] * NPAIR
            w2t = [None] * KC
            w3t = [None] * KC
            prt = [[None] * 4 for _ in range(2)]
            xft_n = [None] * NCHUNK  # xft_n[n][kk]

            def load_w23(classes):
                for k in classes:
                    t = wpool.tile([2 * H1, HID], BF16, tag=f"w2_{k}",
                                   name=f"w2_{k}")
                    nc.sync.dma_start(out=t, in_=w2_d[k])
                    w2t[k] = t
                    t = wpool.tile([HID, L], BF16, tag=f"w3_{k}", name=f"w3_{k}")
                    nc.sync.dma_start(out=t, in_=w3_d[k])
                    w3t[k] = t

            def load_pr(g):
                for j in range(2):
                    t = wpool.tile([128, CHUNK], BF16, tag=f"pr_{j}_{g}",
                                   name=f"pr_{j}_{g}")
                    nc.sync.dma_start(out=t, in_=pr_d[j, g])
                    prt[j][g] = t

            def load_group_weights(g, pairs, classes):
                for p in pairs:
                    if w1t[p] is None:
                        w1t[p] = load_w1_pair(p)
                load_w23(classes)
                load_pr(g)

            ones_t = wpool.tile([128, 64], BF16, tag="ones", name="ones_t")
            w1t[0] = load_w1_pair(0)
            xft_n[0] = load_xf_chunk(0)
            w1t[1] = load_w1_pair(1)
            load_w23(GROUPS[0][1])
            nc.sync.dma_start(out=ones_t, in_=ones_d)
            xft_n[1] = load_xf_chunk(1)
            load_pr(0)
            load_group_weights(1, GROUPS[1][0], GROUPS[1][1])
            xft_n[2] = load_xf_chunk(2)
            xft_n[3] = load_xf_chunk(3)
            load_group_weights(2, GROUPS[2][0], GROUPS[2][1])
            # half-class inputs: xfh tiles + its weights (used from phase 2 on)
            xfht = [[None] * NCHUNK_H for _ in range(4)]
            for n in range(NCHUNK_H):
                for kk in range(4):
                    t = xfp.tile([128, CHUNK], BF16, tag=f"xfh_{kk}_{n}",
                                 name=f"xfh_{kk}_{n}")
                    nc.sync.dma_start(
                        out=t,
                        in_=xfh_d[128 * kk:128 * (kk + 1),
                                  CHUNK * n:CHUNK * (n + 1)])
                    xfht[kk][n] = t
            load_group_weights(3, GROUPS[3][0], GROUPS[3][1])
            xft = [[xft_n[n][kk] for n in range(NCHUNK)] for kk in range(4)]

            # ---- main loop ----
            # Software pipeline: chunk n emits its stage-1/2 work with chunk
            # n-1's stage-3/er work interleaved between the stage-2 matmuls,
            # so PE has independent filler during the ACT-gated PSUM waits.
            def emit_back(g, np_, h2s_prev, er_s):
                """stage-3 + er path for chunk np_ using h2s_prev (list of
                per-class h2 tiles). Returns a generator-like list of
                closures so the caller can interleave them."""
                pairs, classes = GROUPS[g]
                nr = 32 * len(classes)
                mr = 16 * len(classes)
                ps3 = s3pool.tile([128, CHUNK], F32, tag="s3p",
                                  name=f"ps3_{g}_{np_}")

                def s3_for(ci):
                    cls = classes[ci]
                    for h in range(2):
                        nc.tensor.matmul(
                            ps3[32 * ci:32 * (ci + 1), 512 * h:512 * (h + 1)],
                            w3t[cls],
                            h2s_prev[ci][:, 512 * h:512 * (h + 1)],
                            start=True, stop=True,
                            tile_position=(0, 32 * ci))

                def finish():
                    lt = ltpool.tile([128, CHUNK], BF16, tag="lt",
                                     name=f"lt_{g}_{np_}")
                    nc.scalar.activation(lt[0:nr, :], ps3[0:nr, :], Tanh)
                    er_ps = erpool.tile([128, CHUNK], F32, tag="erp",
                                        name=f"erps_{g}_{np_}")
                    for j in range(2):
                        d = sqpool.tile([128, CHUNK], BF16, tag="sq",
                                        name=f"d_{g}_{np_}_{j}")
                        nc.vector.tensor_sub(d[0:nr, :], lt[0:nr, :],
                                             prt[j][g][0:nr, :])
                        sq = sqpool.tile([128, CHUNK], BF16, tag="sq",
                                         name=f"sq_{g}_{np_}_{j}")
                        nc.vector.tensor_mul(sq[0:nr, :], d[0:nr, :],
                                             d[0:nr, :])
                        for h in range(2):
                            nc.tensor.matmul(
                                er_ps[64 * j:64 * j + mr,
                                      512 * h:512 * (h + 1)],
                                ones_t[0:nr, 0:mr],
                                sq[0:nr, 512 * h:512 * (h + 1)],
                                start=True, stop=True,
                                tile_position=(0, 64 * j))
                    # er = max(raw * RED, -CLIP) fused into the PSUM->SBUF copy
                    if nr == 128:
                        nc.vector.tensor_scalar(
                            er_s[:, CHUNK * np_:CHUNK * (np_ + 1)],
                            er_ps[:, :],
                            RED, -CLIP,
                            op0=mybir.AluOpType.mult, op1=mybir.AluOpType.max)
                    else:
                        for j in range(2):
                            nc.vector.tensor_scalar(
                                er_s[64 * j:64 * j + mr,
                                     CHUNK * np_:CHUNK * (np_ + 1)],
                                er_ps[64 * j:64 * j + mr, :],
                                RED, -CLIP,
                                op0=mybir.AluOpType.mult, op1=mybir.AluOpType.max)
                    # per-chunk output DMA keeps the tail short
                    cs = slice(CHUNK * np_, CHUNK * (np_ + 1))
                    if g < 3:
                        d0 = er0_d[4 * g:4 * g + len(classes), cs]
                        d1 = er1_d[4 * g:4 * g + len(classes), cs]
                    else:
                        d0 = erh0_d[:, cs]
                        d1 = erh1_d[:, cs]
                    nc.sync.dma_start(
                        out=d0, in_=er_s[0:16 * len(classes):16, cs])
                    nc.sync.dma_start(
                        out=d1, in_=er_s[64:64 + 16 * len(classes):16, cs])

                return s3_for, finish

            def chunk_front(g, n):
                """stage-1 + stage-2 for chunk n; returns h2 tiles."""
                pairs, classes = GROUPS[g]
                h1s = []
                xsrc = xft if g < 3 else xfht
                for pair in pairs:
                    ps1 = mmpool.tile([128, CHUNK], F32, tag="mm",
                                      name=f"ps1_{g}_{n}_{pair}")
                    for kk in range(4):
                        for h in range(2):
                            nc.tensor.matmul(
                                ps1[:, 512 * h:512 * (h + 1)],
                                w1t[pair][kk],
                                xsrc[kk][n][:, 512 * h:512 * (h + 1)],
                                start=(kk == 0), stop=(kk == 3))
                    h1 = h1pool.tile([128, CHUNK], BF16, tag="h1",
                                     name=f"h1_{g}_{n}_{pair}")
                    nc.scalar.activation(h1, ps1, Tanh)
                    h1s.append(h1)
                return h1s

            def s2_class(g, n, ci, h1s):
                pairs, classes = GROUPS[g]
                cls = classes[ci]
                ps2 = mmpool.tile([128, CHUNK], F32, tag="mm",
                                  name=f"ps2_{g}_{n}_{ci}")
                hsrc = h1s[ci // 2]
                off = H1 * (ci % 2)
                for h in range(2):
                    nc.tensor.matmul(
                        ps2[:, 512 * h:512 * (h + 1)],
                        w2t[cls][off:off + H1, :],
                        hsrc[off:off + H1, 512 * h:512 * (h + 1)],
                        start=True, stop=True)
                h2 = h2pool.tile([128, CHUNK], BF16, tag="h2",
                                 name=f"h2_{g}_{n}_{ci}")
                nc.scalar.activation(h2, ps2, Tanh)
                return h2

            # rows 0:64 of er_s = er (proto0, class ci at row 16*ci), rows
            # 64:128 = er1 (proto1); 16x row replication from the
            # ones-matmul, only every 16th row is DMA'd out
            loop_cm = (tc.For_i(0, reps, 1,
                                hint_engines=(mybir.EngineType.PE,
                                              mybir.EngineType.Activation,
                                              mybir.EngineType.DVE,
                                              mybir.EngineType.SP))
                       if reps > 1 else contextlib.nullcontext())
            with loop_cm:
              for phase in ([0, 1], [2, 3]):
                ers = {g: erspool.tile([128, NB if g < 3 else NH], F32,
                                       tag="ers", name=f"ers_{g}")
                       for g in phase}
                for n in range(NCHUNK):
                    for g in phase:
                        if n >= G_NCH[g]:
                            continue
                        classes = GROUPS[g][1]
                        h1s = chunk_front(g, n)
                        h2s = [s2_class(g, n, ci, h1s)
                               for ci in range(len(classes))]
                        back = emit_back(g, n, h2s, ers[g])
                        for ci in range(len(classes)):
                            back[0](ci)
                        back[1]()

    nc.compile()
    return nc


def _prep_in_maps(x, W1, W2, W3, prototypes, prototypes1):
    bf16 = ml_dtypes.bfloat16
    KPAD = NCORES * KC

    x = np.asarray(x, np.float32)
    xf = np.ascontiguousarray(
        x.reshape(B, C, P).transpose(1, 0, 2).reshape(C, NB)).astype(bf16)

    def pad_k(a):
        out = np.zeros((KPAD,) + a.shape[1:], np.float32)
        out[:K] = np.asarray(a, np.float32)
        return out

    W1p = pad_k(W1)                       # [104, H1, C]
    W2p = pad_k(W2)                       # [104, HID, H1]
    W3p = pad_k(W3)                       # [104, L, HID]
    Pr0 = pad_k(np.asarray(prototypes, np.float32).reshape(K, L, P))
    Pr1 = pad_k(np.asarray(prototypes1, np.float32).reshape(K, L, P))

    ones_blk = np.zeros((128, 64), bf16)
    for m in range(64):
        ones_blk[32 * (m // 16):32 * (m // 16) + 32, m] = 1.0

    in_maps = []
    for c in range(NCORES):
        # slot classes: 12 full + this core's leftover (batch-half) class
        slots = list(range(c * KF, (c + 1) * KF)) + [NCORES * KF + c // 2]
        bhalf = c % 2
        xfh = np.ascontiguousarray(xf[:, NH * bhalf:NH * (bhalf + 1)])
        w1c = W1p[slots].transpose(0, 2, 1)   # [13, C, H1]
        w1c = np.concatenate([w1c, np.zeros((1, C, H1), np.float32)], 0)
        w1pair = np.ascontiguousarray(
            w1c.reshape(NPAIR, 2, C, H1).transpose(0, 2, 1, 3)
            .reshape(NPAIR, C, 2 * H1)).astype(bf16)
        w2c = np.ascontiguousarray(W2p[slots].transpose(0, 2, 1)).astype(bf16)
        w2c = np.concatenate([w2c, w2c], axis=1)          # [13, 128, HID]
        w3c = np.ascontiguousarray(W3p[slots].transpose(0, 2, 1)).astype(bf16)
        prc = np.zeros((2, 4, 128, CHUNK), np.float32)
        for srcfull, j in ((Pr0, 0), (Pr1, 1)):
            src = srcfull[slots]
            for g in range(4):
                for jj in range(4):
                    ks = 4 * g + jj
                    if ks < KC:
                        # tile the [L, P] pattern across the 16 b's of a chunk
                        prc[j, g, 32 * jj:32 * (jj + 1)] = np.tile(
                            src[ks], (1, CHUNK // P))
        in_maps.append({
            "xf": xf,
            "xfh": xfh,
            "w1p": w1pair,
            "w2t": w2c,
            "w3t": w3c,
            "prb": prc.astype(bf16),
            "onesb": ones_blk,
        })
    return in_maps


def _assemble(results, ycls):
    # device already applied  er = max(raw * RED, -CLIP)
    # full classes 0..95: 12 rows per core; leftover classes 96..99: two
    # batch-halves (2048 columns each) from cores (2j, 2j+1)
    er0 = np.concatenate(
        [np.concatenate([r["er0"] for r in results], 0)] +
        [np.concatenate([results[2 * j]["erh0"],
                         results[2 * j + 1]["erh0"]], 1) for j in range(4)],
        0)                                                     # [100, 4096]
    er1 = np.concatenate(
        [np.concatenate([r["er1"] for r in results], 0)] +
        [np.concatenate([results[2 * j]["erh1"],
                         results[2 * j + 1]["erh1"]], 1) for j in range(4)],
        0)

    logits = np.ascontiguousarray(
        er0.reshape(K, B, P).transpose(1, 0, 2).reshape(B, K, H, W))
    logits1 = np.ascontiguousarray(
        er1.reshape(K, B, P).transpose(1, 0, 2).reshape(B, K, H, W))

    f = er0.reshape(K, B, P).sum(axis=2, dtype=np.float32)     # [K, B]
    f1 = er1.reshape(K, B, P).sum(axis=2, dtype=np.float32)
    ycls = np.asarray(ycls)
    mask_eq = (ycls[None, :] == np.arange(K)[:, None]).astype(np.float32)
    n_eq = mask_eq.sum(axis=1)
    pull = np.where(n_eq > 0, (f1 * mask_eq).sum(axis=1) / np.maximum(n_eq, 1.0),
                    0.0).sum(dtype=np.float32)
    comb = (1.0 - mask_eq) * (f < PUSH_THRESH)
    n_comb = comb.sum(axis=1)
    push = np.where(n_comb > 0, (f * comb).sum(axis=1) / np.maximum(n_comb, 1.0),
                    0.0).sum(dtype=np.float32)
    return logits, logits1, np.float32(pull), np.float32(push)


def kernel_ex(inputs, trace=False):
    """Run the bass kernel; returns ((logits, logits1, pull, push), exec_time_ns)."""
    nc = _CACHE.get("nc")
    if nc is None:
        nc = _build_program()
        _CACHE["nc"] = nc
    in_maps = _prep_in_maps(inputs["x"], inputs["W1"], inputs["W2"],
                            inputs["W3"], inputs["prototypes"],
                            inputs["prototypes1"])
    try:
        res = bass_utils.run_bass_kernel_spmd(
            nc, in_maps, core_ids=list(range(NCORES)), trace=trace)
    except ModuleNotFoundError:
        # BASS_TRACE in the env but no axon NTFF hook module available here
        os.environ["BASS_NEVER_TRACE"] = "1"
        res = bass_utils.run_bass_kernel_spmd(
            nc, in_maps, core_ids=list(range(NCORES)), trace=False)
    outs = _assemble(res.results, inputs["ycls"])
    return outs, res.exec_time_ns


def kernel(**inputs):
    outs, _ = kernel_ex(inputs, trace=False)
    return outs


# revision 71
# speedup vs baseline: 1.2247x; 1.0591x over previous
"""CSSR classifier kernel for 8 Trainium2 NeuronCores.

Math (per class k):
    h1 = tanh(W1[k] @ xf)          xf: [C=512, B*P=4096]
    h2 = tanh(W2[k] @ h1)
    lt = tanh(W3[k] @ h2)          [L=32, B*P]
    er_raw  = sum_l (lt - proto )^2     -> [B*P]
    er_raw1 = sum_l (lt - proto1)^2
Device returns er_raw per class; host applies  er = max(-0.1*er_raw, -100),
assembles logits and computes the scalar pull/push losses (O(K*B) work).

Sharding: class dim K=100 -> 8 cores x 13 class slots (104, last 4 dummy).
x is replicated; each core holds only its slice of W1/W2/W3/prototypes.
"""

import contextlib
import os
import sys

if "/opt/trn_rl_repo" not in sys.path:
    sys.path.insert(0, "/opt/trn_rl_repo")

import numpy as np
import ml_dtypes

import concourse.bacc as bacc
import concourse.mybir as mybir
import concourse.tile as tile
from concourse import bass_utils

BF16 = mybir.dt.bfloat16
F32 = mybir.dt.float32
Tanh = mybir.ActivationFunctionType.Tanh

# problem dims (hardcoded per contract)
B, C, H, W = 64, 512, 8, 8
P = H * W                  # 64 spatial positions
K = 100                    # classes
H1, HID, L = 64, 128, 32
CLIP = 100.0
RED = -0.1
PUSH_THRESH = 10000.0

NCORES = 8
# Load balance: 100 = 8*12.5. Each core gets 12 full classes; the 4 leftover
# classes (96..99) are batch-split, half (B=32 -> 2048 columns) on each of
# two cores. Slot 12 of every core is its half-width leftover class.
KF = 12                    # full classes per core
KC = 13                    # weight slots per core (12 full + 1 half)
NPAIR = 7                  # stage-1 pairs (pairs 0-5 full, pair 6 = leftover+dummy)
NB = B * P                 # 4096 free columns
CHUNK = 1024
NCHUNK = NB // CHUNK       # 4
NH = NB // 2               # 2048 columns for the half class
NCHUNK_H = NH // CHUNK     # 2

# groups of <=4 classes that share one stage-3/er pack; group 3 is the
# half-width leftover class
GROUPS = [
    ([0, 1], [0, 1, 2, 3]),
    ([2, 3], [4, 5, 6, 7]),
    ([4, 5], [8, 9, 10, 11]),
    ([6], [12]),
]
G_NCH = [NCHUNK, NCHUNK, NCHUNK, NCHUNK_H]  # chunks per group

_CACHE = {}
PIPELINE = False


def _build_program(reps=1):
    """reps>1 wraps the compute in a hardware loop (timing builds only)."""
    nc = bacc.Bacc("TRN2", target_bir_lowering=False, debug=False)

    xf_d = nc.dram_tensor("xf", [C, NB], BF16, kind="ExternalInput").ap()
    # per-core half of xf for this core's leftover class (its batch half)
    xfh_d = nc.dram_tensor("xfh", [C, NH], BF16, kind="ExternalInput").ap()
    w1_d = nc.dram_tensor("w1p", [NPAIR, C, 2 * H1], BF16, kind="ExternalInput").ap()
    # W2[k].T duplicated vertically so lhsT can be based at partition 0 or 64
    # (matmul requires lhsT and rhs to share a base partition)
    w2_d = nc.dram_tensor("w2t", [KC, 2 * H1, HID], BF16, kind="ExternalInput").ap()
    w3_d = nc.dram_tensor("w3t", [KC, HID, L], BF16, kind="ExternalInput").ap()
    # prototypes pre-broadcast over the 16 b's of a chunk: [2, group, 128, CHUNK]
    pr_d = nc.dram_tensor("prb", [2, 4, 128, CHUNK], BF16, kind="ExternalInput").ap()
    # block "ones" with 16x output replication: ones[l, m] = (l//32 == m//16).
    # The er-reduce matmul then fills all 64 output partitions, so the
    # scale+clip PSUM->SBUF copy is one full-width DVE op per chunk.
    ones_d = nc.dram_tensor("onesb", [128, 64], BF16, kind="ExternalInput").ap()
    er0_d = nc.dram_tensor("er0", [KF, NB], F32, kind="ExternalOutput").ap()
    er1_d = nc.dram_tensor("er1", [KF, NB], F32, kind="ExternalOutput").ap()
    erh0_d = nc.dram_tensor("erh0", [1, NH], F32, kind="ExternalOutput").ap()
    erh1_d = nc.dram_tensor("erh1", [1, NH], F32, kind="ExternalOutput").ap()

    with tile.TileContext(nc) as tc:
        with (
            tc.tile_pool(name="weights", bufs=1) as wpool,
            tc.tile_pool(name="xfp", bufs=1) as xfp,
            tc.tile_pool(name="h1p", bufs=6) as h1pool,
            tc.tile_pool(name="h2p", bufs=8) as h2pool,
            tc.tile_pool(name="ltp", bufs=4) as ltpool,
            tc.tile_pool(name="sqp", bufs=6) as sqpool,
            tc.tile_pool(name="ersp", bufs=3) as erspool,
            tc.tile_pool(name="mmp", bufs=2, space="PSUM") as mmpool,
            tc.tile_pool(name="s3p", bufs=1, space="PSUM") as s3pool,
            tc.tile_pool(name="erp", bufs=1, space="PSUM") as erpool,
        ):
            # ---- DMA issue order matters: the first matmul needs w1[pair0]
            # and the n=0 xf chunk, so those go first; the rest of the
            # weights trail behind, interleaved group-by-group ----
            def load_w1_pair(p):
                tiles = []
                for kk in range(4):
                    t = wpool.tile([128, 2 * H1], BF16, tag=f"w1_{p}_{kk}",
                                   name=f"w1_{p}_{kk}")
                    nc.sync.dma_start(
                        out=t, in_=w1_d[p, 128 * kk:128 * (kk + 1), :])
                    tiles.append(t)
                return tiles

            def load_xf_chunk(n):
                tiles = []
                for kk in range(4):
                    t = xfp.tile([128, CHUNK], BF16, tag=f"xf_{kk}_{n}",
                                 name=f"xf_{kk}_{n}")
                    nc.sync.dma_start(
                        out=t,
                        in_=xf_d[128 * kk:128 * (kk + 1),
                                 CHUNK * n:CHUNK * (n + 1)])
                    tiles.append(t)
                return tiles

            w1t = [None] * NPAIR
            w2t = [None] * KC
            w3t = [None] * KC
            prt = [[None] * 4 for _ in range(2)]
            xft_n = [None] * NCHUNK  # xft_n[n][kk]

            def load_w23(classes):
                for k in classes:
                    t = wpool.tile([2 * H1, HID], BF16, tag=f"w2_{k}",
                                   name=f"w2_{k}")
                    nc.sync.dma_start(out=t, in_=w2_d[k])
                    w2t[k] = t
                    t = wpool.tile([HID, L], BF16, tag=f"w3_{k}", name=f"w3_{k}")
                    nc.sync.dma_start(out=t, in_=w3_d[k])
                    w3t[k] = t

            def load_pr(g):
                for j in range(2):
                    t = wpool.tile([128, CHUNK], BF16, tag=f"pr_{j}_{g}",
                                   name=f"pr_{j}_{g}")
                    nc.sync.dma_start(out=t, in_=pr_d[j, g])
                    prt[j][g] = t

            def load_group_weights(g, pairs, classes):
                for p in pairs:
                    if w1t[p] is None:
                        w1t[p] = load_w1_pair(p)
                load_w23(classes)
                load_pr(g)

            ones_t = wpool.tile([128, 64], BF16, tag="ones", name="ones_t")
            w1t[0] = load_w1_pair(0)
            xft_n[0] = load_xf_chunk(0)
            w1t[1] = load_w1_pair(1)
            load_w23(GROUPS[0][1])
            nc.sync.dma_start(out=ones_t, in_=ones_d)
            # phase 0 interleaves groups 0 and 1 from chunk 0, so group 1's
            # stage-1/2 weights must land before the bulky xf chunk 1
            w1t[2] = load_w1_pair(2)
            w1t[3] = load_w1_pair(3)
            load_w23(GROUPS[1][1])
            xft_n[1] = load_xf_chunk(1)
            load_pr(0)
            load_pr(1)
            xft_n[2] = load_xf_chunk(2)
            xft_n[3] = load_xf_chunk(3)
            load_group_weights(2, GROUPS[2][0], GROUPS[2][1])
            # half-class inputs: xfh tiles + its weights (used from phase 2 on)
            xfht = [[None] * NCHUNK_H for _ in range(4)]
            for n in range(NCHUNK_H):
                for kk in range(4):
                    t = xfp.tile([128, CHUNK], BF16, tag=f"xfh_{kk}_{n}",
                                 name=f"xfh_{kk}_{n}")
                    nc.sync.dma_start(
                        out=t,
                        in_=xfh_d[128 * kk:128 * (kk + 1),
                                  CHUNK * n:CHUNK * (n + 1)])
                    xfht[kk][n] = t
            load_group_weights(3, GROUPS[3][0], GROUPS[3][1])
            xft = [[xft_n[n][kk] for n in range(NCHUNK)] for kk in range(4)]

            # ---- main loop ----
            # Software pipeline: chunk n emits its stage-1/2 work with chunk
            # n-1's stage-3/er work interleaved between the stage-2 matmuls,
            # so PE has independent filler during the ACT-gated PSUM waits.
            def emit_back(g, np_, h2s_prev, er_s):
                """stage-3 + er path for chunk np_ using h2s_prev (list of
                per-class h2 tiles). Returns a generator-like list of
                closures so the caller can interleave them."""
                pairs, classes = GROUPS[g]
                nr = 32 * len(classes)
                mr = 16 * len(classes)
                ps3 = s3pool.tile([128, CHUNK], F32, tag="s3p",
                                  name=f"ps3_{g}_{np_}")

                def s3_for(ci):
                    cls = classes[ci]
                    for h in range(2):
                        nc.tensor.matmul(
                            ps3[32 * ci:32 * (ci + 1), 512 * h:512 * (h + 1)],
                            w3t[cls],
                            h2s_prev[ci][:, 512 * h:512 * (h + 1)],
                            start=True, stop=True,
                            tile_position=(0, 32 * ci))

                def finish():
                    lt = ltpool.tile([128, CHUNK], BF16, tag="lt",
                                     name=f"lt_{g}_{np_}")
                    nc.scalar.activation(lt[0:nr, :], ps3[0:nr, :], Tanh)
                    er_ps = erpool.tile([128, CHUNK], F32, tag="erp",
                                        name=f"erps_{g}_{np_}")
                    for j in range(2):
                        d = sqpool.tile([128, CHUNK], BF16, tag="sq",
                                        name=f"d_{g}_{np_}_{j}")
                        nc.vector.tensor_sub(d[0:nr, :], lt[0:nr, :],
                                             prt[j][g][0:nr, :])
                        sq = sqpool.tile([128, CHUNK], BF16, tag="sq",
                                         name=f"sq_{g}_{np_}_{j}")
                        nc.vector.tensor_mul(sq[0:nr, :], d[0:nr, :],
                                             d[0:nr, :])
                        for h in range(2):
                            nc.tensor.matmul(
                                er_ps[64 * j:64 * j + mr,
                                      512 * h:512 * (h + 1)],
                                ones_t[0:nr, 0:mr],
                                sq[0:nr, 512 * h:512 * (h + 1)],
                                start=True, stop=True,
                                tile_position=(0, 64 * j))
                    # er = max(raw * RED, -CLIP) fused into the PSUM->SBUF copy
                    if nr == 128:
                        nc.vector.tensor_scalar(
                            er_s[:, CHUNK * np_:CHUNK * (np_ + 1)],
                            er_ps[:, :],
                            RED, -CLIP,
                            op0=mybir.AluOpType.mult, op1=mybir.AluOpType.max)
                    else:
                        for j in range(2):
                            nc.vector.tensor_scalar(
                                er_s[64 * j:64 * j + mr,
                                     CHUNK * np_:CHUNK * (np_ + 1)],
                                er_ps[64 * j:64 * j + mr, :],
                                RED, -CLIP,
                                op0=mybir.AluOpType.mult, op1=mybir.AluOpType.max)
                    # per-chunk output DMA keeps the tail short
                    cs = slice(CHUNK * np_, CHUNK * (np_ + 1))
                    if g < 3:
                        d0 = er0_d[4 * g:4 * g + len(classes), cs]
                        d1 = er1_d[4 * g:4 * g + len(classes), cs]
                    else:
                        d0 = erh0_d[:, cs]
                        d1 = erh1_d[:, cs]
                    nc.sync.dma_start(
                        out=d0, in_=er_s[0:16 * len(classes):16, cs])
                    nc.sync.dma_start(
                        out=d1, in_=er_s[64:64 + 16 * len(classes):16, cs])

                return s3_for, finish

            def chunk_front(g, n):
                """stage-1 + stage-2 for chunk n; returns h2 tiles."""
                pairs, classes = GROUPS[g]
                h1s = []
                xsrc = xft if g < 3 else xfht
                for pair in pairs:
                    ps1 = mmpool.tile([128, CHUNK], F32, tag="mm",
                                      name=f"ps1_{g}_{n}_{pair}")
                    for kk in range(4):
                        for h in range(2):
                            nc.tensor.matmul(
                                ps1[:, 512 * h:512 * (h + 1)],
                                w1t[pair][kk],
                                xsrc[kk][n][:, 512 * h:512 * (h + 1)],
                                start=(kk == 0), stop=(kk == 3))
                    h1 = h1pool.tile([128, CHUNK], BF16, tag="h1",
                                     name=f"h1_{g}_{n}_{pair}")
                    nc.scalar.activation(h1, ps1, Tanh)
                    h1s.append(h1)
                return h1s

            def s2_class(g, n, ci, h1s):
                pairs, classes = GROUPS[g]
                cls = classes[ci]
                ps2 = mmpool.tile([128, CHUNK], F32, tag="mm",
                                  name=f"ps2_{g}_{n}_{ci}")
                hsrc = h1s[ci // 2]
                off = H1 * (ci % 2)
                for h in range(2):
                    nc.tensor.matmul(
                        ps2[:, 512 * h:512 * (h + 1)],
                        w2t[cls][off:off + H1, :],
                        hsrc[off:off + H1, 512 * h:512 * (h + 1)],
                        start=True, stop=True)
                h2 = h2pool.tile([128, CHUNK], BF16, tag="h2",
                                 name=f"h2_{g}_{n}_{ci}")
                nc.scalar.activation(h2, ps2, Tanh)
                return h2

            # rows 0:64 of er_s = er (proto0, class ci at row 16*ci), rows
            # 64:128 = er1 (proto1); 16x row replication from the
            # ones-matmul, only every 16th row is DMA'd out
            loop_cm = (tc.For_i(0, reps, 1,
                                hint_engines=(mybir.EngineType.PE,
                                              mybir.EngineType.Activation,
                                              mybir.EngineType.DVE,
                                              mybir.EngineType.SP))
                       if reps > 1 else contextlib.nullcontext())
            with loop_cm:
              for phase in ([0, 1], [2, 3]):
                ers = {g: erspool.tile([128, NB if g < 3 else NH], F32,
                                       tag="ers", name=f"ers_{g}")
                       for g in phase}
                for n in range(NCHUNK):
                    for g in phase:
                        # short groups (the half-width leftover class) run in
                        # the LAST chunks of the phase, filling the tail;
                        # nl is the group-local chunk index
                        nl = n - (NCHUNK - G_NCH[g])
                        if nl < 0:
                            continue
                        classes = GROUPS[g][1]
                        h1s = chunk_front(g, nl)
                        h2s = [s2_class(g, nl, ci, h1s)
                               for ci in range(len(classes))]
                        back = emit_back(g, nl, h2s, ers[g])
                        for ci in range(len(classes)):
                            back[0](ci)
                        back[1]()

    nc.compile()
    return nc


def _prep_in_maps(x, W1, W2, W3, prototypes, prototypes1):
    bf16 = ml_dtypes.bfloat16
    KPAD = NCORES * KC

    x = np.asarray(x, np.float32)
    xf = np.ascontiguousarray(
        x.reshape(B, C, P).transpose(1, 0, 2).reshape(C, NB)).astype(bf16)

    def pad_k(a):
        out = np.zeros((KPAD,) + a.shape[1:], np.float32)
        out[:K] = np.asarray(a, np.float32)
        return out

    W1p = pad_k(W1)                       # [104, H1, C]
    W2p = pad_k(W2)                       # [104, HID, H1]
    W3p = pad_k(W3)                       # [104, L, HID]
    Pr0 = pad_k(np.asarray(prototypes, np.float32).reshape(K, L, P))
    Pr1 = pad_k(np.asarray(prototypes1, np.float32).reshape(K, L, P))

    ones_blk = np.zeros((128, 64), bf16)
    for m in range(64):
        ones_blk[32 * (m // 16):32 * (m // 16) + 32, m] = 1.0

    in_maps = []
    for c in range(NCORES):
        # slot classes: 12 full + this core's leftover (batch-half) class
        slots = list(range(c * KF, (c + 1) * KF)) + [NCORES * KF + c // 2]
        bhalf = c % 2
        xfh = np.ascontiguousarray(xf[:, NH * bhalf:NH * (bhalf + 1)])
        w1c = W1p[slots].transpose(0, 2, 1)   # [13, C, H1]
        w1c = np.concatenate([w1c, np.zeros((1, C, H1), np.float32)], 0)
        w1pair = np.ascontiguousarray(
            w1c.reshape(NPAIR, 2, C, H1).transpose(0, 2, 1, 3)
            .reshape(NPAIR, C, 2 * H1)).astype(bf16)
        w2c = np.ascontiguousarray(W2p[slots].transpose(0, 2, 1)).astype(bf16)
        w2c = np.concatenate([w2c, w2c], axis=1)          # [13, 128, HID]
        w3c = np.ascontiguousarray(W3p[slots].transpose(0, 2, 1)).astype(bf16)
        prc = np.zeros((2, 4, 128, CHUNK), np.float32)
        for srcfull, j in ((Pr0, 0), (Pr1, 1)):
            src = srcfull[slots]
            for g in range(4):
                for jj in range(4):
                    ks = 4 * g + jj
                    if ks < KC:
                        # tile the [L, P] pattern across the 16 b's of a chunk
                        prc[j, g, 32 * jj:32 * (jj + 1)] = np.tile(
                            src[ks], (1, CHUNK // P))
        in_maps.append({
            "xf": xf,
            "xfh": xfh,
            "w1p": w1pair,
            "w2t": w2c,
            "w3t": w3c,
            "prb": prc.astype(bf16),
            "onesb": ones_blk,
        })
    return in_maps


def _assemble(results, ycls):
    # device already applied  er = max(raw * RED, -CLIP)
    # full classes 0..95: 12 rows per core; leftover classes 96..99: two
    # batch-halves (2048 columns each) from cores (2j, 2j+1)
    er0 = np.concatenate(
        [np.concatenate([r["er0"] for r in results], 0)] +
        [np.concatenate([results[2 * j]["erh0"],
                         results[2 * j + 1]["erh0"]], 1) for j in range(4)],
        0)                                                     # [100, 4096]
    er1 = np.concatenate(
        [np.concatenate([r["er1"] for r in results], 0)] +
        [np.concatenate([results[2 * j]["erh1"],
                         results[2 * j + 1]["erh1"]], 1) for j in range(4)],
        0)

    logits = np.ascontiguousarray(
        er0.reshape(K, B, P).transpose(1, 0, 2).reshape(B, K, H, W))
    logits1 = np.ascontiguousarray(
        er1.reshape(K, B, P).transpose(1, 0, 2).reshape(B, K, H, W))

    f = er0.reshape(K, B, P).sum(axis=2, dtype=np.float32)     # [K, B]
    f1 = er1.reshape(K, B, P).sum(axis=2, dtype=np.float32)
    ycls = np.asarray(ycls)
    mask_eq = (ycls[None, :] == np.arange(K)[:, None]).astype(np.float32)
    n_eq = mask_eq.sum(axis=1)
    pull = np.where(n_eq > 0, (f1 * mask_eq).sum(axis=1) / np.maximum(n_eq, 1.0),
                    0.0).sum(dtype=np.float32)
    comb = (1.0 - mask_eq) * (f < PUSH_THRESH)
    n_comb = comb.sum(axis=1)
    push = np.where(n_comb > 0, (f * comb).sum(axis=1) / np.maximum(n_comb, 1.0),
                    0.0).sum(dtype=np.float32)
    return logits, logits1, np.float32(pull), np.float32(push)


def kernel_ex(inputs, trace=False):
    """Run the bass kernel; returns ((logits, logits1, pull, push), exec_time_ns)."""
    nc = _CACHE.get("nc")
    if nc is None:
        nc = _build_program()
        _CACHE["nc"] = nc
    in_maps = _prep_in_maps(inputs["x"], inputs["W1"], inputs["W2"],
                            inputs["W3"], inputs["prototypes"],
                            inputs["prototypes1"])
    try:
        res = bass_utils.run_bass_kernel_spmd(
            nc, in_maps, core_ids=list(range(NCORES)), trace=trace)
    except ModuleNotFoundError:
        # BASS_TRACE in the env but no axon NTFF hook module available here
        os.environ["BASS_NEVER_TRACE"] = "1"
        res = bass_utils.run_bass_kernel_spmd(
            nc, in_maps, core_ids=list(range(NCORES)), trace=False)
    outs = _assemble(res.results, inputs["ycls"])
    return outs, res.exec_time_ns


def kernel(**inputs):
    outs, _ = kernel_ex(inputs, trace=False)
    return outs


# revision 73
# speedup vs baseline: 1.5712x; 1.2830x over previous
"""CSSR classifier kernel for 8 Trainium2 NeuronCores.

Math (per class k):
    h1 = tanh(W1[k] @ xf)          xf: [C=512, B*P=4096]
    h2 = tanh(W2[k] @ h1)
    lt = tanh(W3[k] @ h2)          [L=32, B*P]
    er_raw  = sum_l (lt - proto )^2     -> [B*P]
    er_raw1 = sum_l (lt - proto1)^2
Device returns er_raw per class; host applies  er = max(-0.1*er_raw, -100),
assembles logits and computes the scalar pull/push losses (O(K*B) work).

Sharding: class dim K=100 -> 8 cores x 13 class slots (104, last 4 dummy).
x is replicated; each core holds only its slice of W1/W2/W3/prototypes.
"""

import contextlib
import os
import sys

if "/opt/trn_rl_repo" not in sys.path:
    sys.path.insert(0, "/opt/trn_rl_repo")

import numpy as np
import ml_dtypes

import concourse.bacc as bacc
import concourse.mybir as mybir
import concourse.tile as tile
from concourse import bass_utils

BF16 = mybir.dt.bfloat16
F32 = mybir.dt.float32
Tanh = mybir.ActivationFunctionType.Tanh

# problem dims (hardcoded per contract)
B, C, H, W = 64, 512, 8, 8
P = H * W                  # 64 spatial positions
K = 100                    # classes
H1, HID, L = 64, 128, 32
CLIP = 100.0
RED = -0.1
PUSH_THRESH = 10000.0

NCORES = 8
# Load balance: 100 = 8*12.5. Each core gets 12 full classes; the 4 leftover
# classes (96..99) are batch-split, half (B=32 -> 2048 columns) on each of
# two cores. Slot 12 of every core is its half-width leftover class.
KF = 12                    # full classes per core
KC = 13                    # weight slots per core (12 full + 1 half)
NPAIR = 7                  # stage-1 pairs (pairs 0-5 full, pair 6 = leftover+dummy)
NB = B * P                 # 4096 free columns
CHUNK = 1024
NCHUNK = NB // CHUNK       # 4
NH = NB // 2               # 2048 columns for the half class
NCHUNK_H = NH // CHUNK     # 2

# groups of <=4 classes that share one stage-3/er pack; group 3 is the
# half-width leftover class
GROUPS = [
    ([0, 1], [0, 1, 2, 3]),
    ([2, 3], [4, 5, 6, 7]),
    ([4, 5], [8, 9, 10, 11]),
    ([6], [12]),
]
G_NCH = [NCHUNK, NCHUNK, NCHUNK, NCHUNK_H]  # chunks per group

_CACHE = {}
PIPELINE = False


def _build_program(reps=1):
    """reps>1 wraps the compute in a hardware loop (timing builds only)."""
    nc = bacc.Bacc("TRN2", target_bir_lowering=False, debug=False)

    xf_d = nc.dram_tensor("xf", [C, NB], BF16, kind="ExternalInput").ap()
    # per-core half of xf for this core's leftover class (its batch half)
    xfh_d = nc.dram_tensor("xfh", [C, NH], BF16, kind="ExternalInput").ap()
    w1_d = nc.dram_tensor("w1p", [NPAIR, C, 2 * H1], BF16, kind="ExternalInput").ap()
    # W2[k].T duplicated vertically so lhsT can be based at partition 0 or 64
    # (matmul requires lhsT and rhs to share a base partition)
    w2_d = nc.dram_tensor("w2t", [KC, 2 * H1, HID], BF16, kind="ExternalInput").ap()
    w3_d = nc.dram_tensor("w3t", [KC, HID, L], BF16, kind="ExternalInput").ap()
    # prototypes pre-broadcast over the 16 b's of a chunk: [2, group, 128, CHUNK]
    pr_d = nc.dram_tensor("prb", [2, 4, 128, CHUNK], BF16, kind="ExternalInput").ap()
    # block "ones" with 16x output replication: ones[l, m] = (l//32 == m//16).
    # The er-reduce matmul then fills all 64 output partitions, so the
    # scale+clip PSUM->SBUF copy is one full-width DVE op per chunk.
    ones_d = nc.dram_tensor("onesb", [128, 64], BF16, kind="ExternalInput").ap()
    er0_d = nc.dram_tensor("er0", [KF, NB], F32, kind="ExternalOutput").ap()
    er1_d = nc.dram_tensor("er1", [KF, NB], F32, kind="ExternalOutput").ap()
    erh0_d = nc.dram_tensor("erh0", [1, NH], F32, kind="ExternalOutput").ap()
    erh1_d = nc.dram_tensor("erh1", [1, NH], F32, kind="ExternalOutput").ap()

    with tile.TileContext(nc) as tc:
        with (
            tc.tile_pool(name="weights", bufs=1) as wpool,
            tc.tile_pool(name="xfp", bufs=1) as xfp,
            tc.tile_pool(name="h1p", bufs=6) as h1pool,
            tc.tile_pool(name="h2p", bufs=8) as h2pool,
            tc.tile_pool(name="ltp", bufs=4) as ltpool,
            tc.tile_pool(name="sqp", bufs=6) as sqpool,
            tc.tile_pool(name="ersp", bufs=3) as erspool,
            tc.tile_pool(name="mmp", bufs=2, space="PSUM") as mmpool,
            tc.tile_pool(name="s3p", bufs=1, space="PSUM") as s3pool,
            tc.tile_pool(name="erp", bufs=1, space="PSUM") as erpool,
        ):
            # ---- DMA issue order matters: the first matmul needs w1[pair0]
            # and the n=0 xf chunk, so those go first; the rest of the
            # weights trail behind, interleaved group-by-group ----
            def load_w1_pair(p):
                tiles = []
                for kk in range(4):
                    t = wpool.tile([128, 2 * H1], BF16, tag=f"w1_{p}_{kk}",
                                   name=f"w1_{p}_{kk}")
                    nc.sync.dma_start(
                        out=t, in_=w1_d[p, 128 * kk:128 * (kk + 1), :])
                    tiles.append(t)
                return tiles

            def load_xf_chunk(n):
                tiles = []
                for kk in range(4):
                    t = xfp.tile([128, CHUNK], BF16, tag=f"xf_{kk}_{n}",
                                 name=f"xf_{kk}_{n}")
                    nc.sync.dma_start(
                        out=t,
                        in_=xf_d[128 * kk:128 * (kk + 1),
                                 CHUNK * n:CHUNK * (n + 1)])
                    tiles.append(t)
                return tiles

            w1t = [None] * NPAIR
            w2t = [None] * KC
            w3t = [None] * KC
            prt = [[None] * 4 for _ in range(2)]
            xft_n = [None] * NCHUNK  # xft_n[n][kk]

            def load_w23(classes):
                for k in classes:
                    t = wpool.tile([2 * H1, HID], BF16, tag=f"w2_{k}",
                                   name=f"w2_{k}")
                    nc.sync.dma_start(out=t, in_=w2_d[k])
                    w2t[k] = t
                    t = wpool.tile([HID, L], BF16, tag=f"w3_{k}", name=f"w3_{k}")
                    nc.sync.dma_start(out=t, in_=w3_d[k])
                    w3t[k] = t

            def load_pr(g):
                for j in range(2):
                    t = wpool.tile([128, CHUNK], BF16, tag=f"pr_{j}_{g}",
                                   name=f"pr_{j}_{g}")
                    nc.sync.dma_start(out=t, in_=pr_d[j, g])
                    prt[j][g] = t

            def load_group_weights(g, pairs, classes):
                for p in pairs:
                    if w1t[p] is None:
                        w1t[p] = load_w1_pair(p)
                load_w23(classes)
                load_pr(g)

            ones_t = wpool.tile([128, 64], BF16, tag="ones", name="ones_t")
            w1t[0] = load_w1_pair(0)
            xft_n[0] = load_xf_chunk(0)
            w1t[1] = load_w1_pair(1)
            load_w23(GROUPS[0][1])
            nc.sync.dma_start(out=ones_t, in_=ones_d)
            # phase 0 interleaves groups 0 and 1 from chunk 0, so group 1's
            # stage-1/2 weights must land before the bulky xf chunk 1
            w1t[2] = load_w1_pair(2)
            w1t[3] = load_w1_pair(3)
            load_w23(GROUPS[1][1])
            xft_n[1] = load_xf_chunk(1)
            load_pr(0)
            load_pr(1)
            xft_n[2] = load_xf_chunk(2)
            xft_n[3] = load_xf_chunk(3)
            load_group_weights(2, GROUPS[2][0], GROUPS[2][1])
            # half-class inputs: xfh tiles + its weights (used from phase 2 on)
            xfht = [[None] * NCHUNK_H for _ in range(4)]
            for n in range(NCHUNK_H):
                for kk in range(4):
                    t = xfp.tile([128, CHUNK], BF16, tag=f"xfh_{kk}_{n}",
                                 name=f"xfh_{kk}_{n}")
                    nc.sync.dma_start(
                        out=t,
                        in_=xfh_d[128 * kk:128 * (kk + 1),
                                  CHUNK * n:CHUNK * (n + 1)])
                    xfht[kk][n] = t
            load_group_weights(3, GROUPS[3][0], GROUPS[3][1])
            xft = [[xft_n[n][kk] for n in range(NCHUNK)] for kk in range(4)]

            # ---- main loop ----
            # Software pipeline: chunk n emits its stage-1/2 work with chunk
            # n-1's stage-3/er work interleaved between the stage-2 matmuls,
            # so PE has independent filler during the ACT-gated PSUM waits.
            def emit_back(g, np_, h2s_prev, er_s):
                """stage-3 + er path for chunk np_ using h2s_prev (list of
                per-class h2 tiles). Returns a generator-like list of
                closures so the caller can interleave them."""
                pairs, classes = GROUPS[g]
                nr = 32 * len(classes)
                mr = 16 * len(classes)
                ps3 = s3pool.tile([128, CHUNK], F32, tag="s3p",
                                  name=f"ps3_{g}_{np_}")

                def s3_for(ci):
                    cls = classes[ci]
                    for h in range(2):
                        nc.tensor.matmul(
                            ps3[32 * ci:32 * (ci + 1), 512 * h:512 * (h + 1)],
                            w3t[cls],
                            h2s_prev[ci][:, 512 * h:512 * (h + 1)],
                            start=True, stop=True,
                            tile_position=(0, 32 * ci))

                def finish():
                    lt = ltpool.tile([128, CHUNK], BF16, tag="lt",
                                     name=f"lt_{g}_{np_}")
                    nc.scalar.activation(lt[0:nr, :], ps3[0:nr, :], Tanh)
                    er_ps = erpool.tile([128, CHUNK], F32, tag="erp",
                                        name=f"erps_{g}_{np_}")
                    for j in range(2):
                        d = sqpool.tile([128, CHUNK], BF16, tag="sq",
                                        name=f"d_{g}_{np_}_{j}")
                        nc.vector.tensor_sub(d[0:nr, :], lt[0:nr, :],
                                             prt[j][g][0:nr, :])
                        sq = sqpool.tile([128, CHUNK], BF16, tag="sq",
                                         name=f"sq_{g}_{np_}_{j}")
                        nc.vector.tensor_mul(sq[0:nr, :], d[0:nr, :],
                                             d[0:nr, :])
                        for h in range(2):
                            nc.tensor.matmul(
                                er_ps[64 * j:64 * j + mr,
                                      512 * h:512 * (h + 1)],
                                ones_t[0:nr, 0:mr],
                                sq[0:nr, 512 * h:512 * (h + 1)],
                                start=True, stop=True,
                                tile_position=(0, 64 * j))
                    # er = max(raw * RED, -CLIP) fused into the PSUM->SBUF copy
                    if nr == 128:
                        nc.vector.tensor_scalar(
                            er_s[:, CHUNK * np_:CHUNK * (np_ + 1)],
                            er_ps[:, :],
                            RED, -CLIP,
                            op0=mybir.AluOpType.mult, op1=mybir.AluOpType.max)
                    else:
                        for j in range(2):
                            nc.vector.tensor_scalar(
                                er_s[64 * j:64 * j + mr,
                                     CHUNK * np_:CHUNK * (np_ + 1)],
                                er_ps[64 * j:64 * j + mr, :],
                                RED, -CLIP,
                                op0=mybir.AluOpType.mult, op1=mybir.AluOpType.max)
                    # per-chunk output DMA keeps the tail short
                    cs = slice(CHUNK * np_, CHUNK * (np_ + 1))
                    if g < 3:
                        d0 = er0_d[4 * g:4 * g + len(classes), cs]
                        d1 = er1_d[4 * g:4 * g + len(classes), cs]
                    else:
                        d0 = erh0_d[:, cs]
                        d1 = erh1_d[:, cs]
                    nc.sync.dma_start(
                        out=d0, in_=er_s[0:16 * len(classes):16, cs])
                    nc.sync.dma_start(
                        out=d1, in_=er_s[64:64 + 16 * len(classes):16, cs])

                return s3_for, finish

            def chunk_front(g, n):
                """stage-1 + stage-2 for chunk n; returns h2 tiles."""
                pairs, classes = GROUPS[g]
                h1s = []
                xsrc = xft if g < 3 else xfht
                for pair in pairs:
                    ps1 = mmpool.tile([128, CHUNK], F32, tag="mm",
                                      name=f"ps1_{g}_{n}_{pair}")
                    for kk in range(4):
                        for h in range(2):
                            nc.tensor.matmul(
                                ps1[:, 512 * h:512 * (h + 1)],
                                w1t[pair][kk],
                                xsrc[kk][n][:, 512 * h:512 * (h + 1)],
                                start=(kk == 0), stop=(kk == 3))
                    h1 = h1pool.tile([128, CHUNK], BF16, tag="h1",
                                     name=f"h1_{g}_{n}_{pair}")
                    nc.scalar.activation(h1, ps1, Tanh)
                    h1s.append(h1)
                return h1s

            def s2_class(g, n, ci, h1s):
                pairs, classes = GROUPS[g]
                cls = classes[ci]
                ps2 = mmpool.tile([128, CHUNK], F32, tag="mm",
                                  name=f"ps2_{g}_{n}_{ci}")
                hsrc = h1s[ci // 2]
                off = H1 * (ci % 2)
                for h in range(2):
                    nc.tensor.matmul(
                        ps2[:, 512 * h:512 * (h + 1)],
                        w2t[cls][off:off + H1, :],
                        hsrc[off:off + H1, 512 * h:512 * (h + 1)],
                        start=True, stop=True)
                h2 = h2pool.tile([128, CHUNK], BF16, tag="h2",
                                 name=f"h2_{g}_{n}_{ci}")
                nc.scalar.activation(h2, ps2, Tanh)
                return h2

            # rows 0:64 of er_s = er (proto0, class ci at row 16*ci), rows
            # 64:128 = er1 (proto1); 16x row replication from the
            # ones-matmul, only every 16th row is DMA'd out
            loop_cm = (tc.For_i(0, reps, 1,
                                hint_engines=(mybir.EngineType.PE,
                                              mybir.EngineType.Activation,
                                              mybir.EngineType.DVE,
                                              mybir.EngineType.SP))
                       if reps > 1 else contextlib.nullcontext())
            with loop_cm:
              for phase in ([0, 1], [2, 3]):
                ers = {g: erspool.tile([128, NB if g < 3 else NH], F32,
                                       tag="ers", name=f"ers_{g}")
                       for g in phase}
                for n in range(NCHUNK):
                    for g in phase:
                        # short groups (the half-width leftover class) run in
                        # the LAST chunks of the phase, filling the tail;
                        # nl is the group-local chunk index
                        nl = n - (NCHUNK - G_NCH[g])
                        if nl < 0:
                            continue
                        classes = GROUPS[g][1]
                        h1s = chunk_front(g, nl)
                        h2s = [s2_class(g, nl, ci, h1s)
                               for ci in range(len(classes))]
                        back = emit_back(g, nl, h2s, ers[g])
                        for ci in range(len(classes)):
                            back[0](ci)
                        back[1]()

    nc.compile()
    return nc


def _prep_in_maps(x, W1, W2, W3, prototypes, prototypes1):
    bf16 = ml_dtypes.bfloat16
    KPAD = NCORES * KC

    x = np.asarray(x, np.float32)
    xf = np.ascontiguousarray(
        x.reshape(B, C, P).transpose(1, 0, 2).reshape(C, NB)).astype(bf16)

    def pad_k(a):
        out = np.zeros((KPAD,) + a.shape[1:], np.float32)
        out[:K] = np.asarray(a, np.float32)
        return out

    W1p = pad_k(W1)                       # [104, H1, C]
    W2p = pad_k(W2)                       # [104, HID, H1]
    W3p = pad_k(W3)                       # [104, L, HID]
    Pr0 = pad_k(np.asarray(prototypes, np.float32).reshape(K, L, P))
    Pr1 = pad_k(np.asarray(prototypes1, np.float32).reshape(K, L, P))

    ones_blk = np.zeros((128, 64), bf16)
    for m in range(64):
        ones_blk[32 * (m // 16):32 * (m // 16) + 32, m] = 1.0

    in_maps = []
    for c in range(NCORES):
        # slot classes: 12 full + this core's leftover (batch-half) class
        slots = list(range(c * KF, (c + 1) * KF)) + [NCORES * KF + c // 2]
        bhalf = c % 2
        xfh = np.ascontiguousarray(xf[:, NH * bhalf:NH * (bhalf + 1)])
        w1c = W1p[slots].transpose(0, 2, 1)   # [13, C, H1]
        w1c = np.concatenate([w1c, np.zeros((1, C, H1), np.float32)], 0)
        w1pair = np.ascontiguousarray(
            w1c.reshape(NPAIR, 2, C, H1).transpose(0, 2, 1, 3)
            .reshape(NPAIR, C, 2 * H1)).astype(bf16)
        w2c = np.ascontiguousarray(W2p[slots].transpose(0, 2, 1)).astype(bf16)
        w2c = np.concatenate([w2c, w2c], axis=1)          # [13, 128, HID]
        w3c = np.ascontiguousarray(W3p[slots].transpose(0, 2, 1)).astype(bf16)
        prc = np.zeros((2, 4, 128, CHUNK), np.float32)
        for srcfull, j in ((Pr0, 0), (Pr1, 1)):
            src = srcfull[slots]
            for g in range(4):
                for jj in range(4):
                    ks = 4 * g + jj
                    if ks < KC:
                        # tile the [L, P] pattern across the 16 b's of a chunk
                        prc[j, g, 32 * jj:32 * (jj + 1)] = np.tile(
                            src[ks], (1, CHUNK // P))
        in_maps.append({
            "xf": xf,
            "xfh": xfh,
            "w1p": w1pair,
            "w2t": w2c,
            "w3t": w3c,
            "prb": prc.astype(bf16),
            "onesb": ones_blk,
        })
    return in_maps


def _assemble(results, ycls):
    # device already applied  er = max(raw * RED, -CLIP)
    # full classes 0..95: 12 rows per core; leftover classes 96..99: two
    # batch-halves (2048 columns each) from cores (2j, 2j+1)
    er0 = np.concatenate(
        [np.concatenate([r["er0"] for r in results], 0)] +
        [np.concatenate([results[2 * j]["erh0"],
                         results[2 * j + 1]["erh0"]], 1) for j in range(4)],
        0)                                                     # [100, 4096]
    er1 = np.concatenate(
        [np.concatenate([r["er1"] for r in results], 0)] +
        [np.concatenate([results[2 * j]["erh1"],
                         results[2 * j + 1]["erh1"]], 1) for j in range(4)],
        0)

    logits = np.ascontiguousarray(
        er0.reshape(K, B, P).transpose(1, 0, 2).reshape(B, K, H, W))
    logits1 = np.ascontiguousarray(
        er1.reshape(K, B, P).transpose(1, 0, 2).reshape(B, K, H, W))

    f = er0.reshape(K, B, P).sum(axis=2, dtype=np.float32)     # [K, B]
    f1 = er1.reshape(K, B, P).sum(axis=2, dtype=np.float32)
    ycls = np.asarray(ycls)
    mask_eq = (ycls[None, :] == np.arange(K)[:, None]).astype(np.float32)
    n_eq = mask_eq.sum(axis=1)
    pull = np.where(n_eq > 0, (f1 * mask_eq).sum(axis=1) / np.maximum(n_eq, 1.0),
                    0.0).sum(dtype=np.float32)
    comb = (1.0 - mask_eq) * (f < PUSH_THRESH)
    n_comb = comb.sum(axis=1)
    push = np.where(n_comb > 0, (f * comb).sum(axis=1) / np.maximum(n_comb, 1.0),
                    0.0).sum(dtype=np.float32)
    return logits, logits1, np.float32(pull), np.float32(push)


def kernel_ex(inputs, trace=False):
    """Run the bass kernel; returns ((logits, logits1, pull, push), exec_time_ns)."""
    nc = _CACHE.get("nc")
    if nc is None:
        nc = _build_program()
        _CACHE["nc"] = nc
    in_maps = _prep_in_maps(inputs["x"], inputs["W1"], inputs["W2"],
                            inputs["W3"], inputs["prototypes"],
                            inputs["prototypes1"])
    try:
        res = bass_utils.run_bass_kernel_spmd(
            nc, in_maps, core_ids=list(range(NCORES)), trace=trace)
    except ModuleNotFoundError:
        # BASS_TRACE in the env but no axon NTFF hook module available here
        os.environ["BASS_NEVER_TRACE"] = "1"
        res = bass_utils.run_bass_kernel_spmd(
            nc, in_maps, core_ids=list(range(NCORES)), trace=False)
    outs = _assemble(res.results, inputs["ycls"])
    return outs, res.exec_time_ns


def kernel(**inputs):
    outs, _ = kernel_ex(inputs, trace=False)
    return outs
